# revision 3
# baseline (speedup 1.0000x reference)
"""Trainium2 Bass kernel for GNN multi-head cross-attention message passing.

Math (see reference): per edge e: score[e,h,g] = qh[A[e],h,:] . kh[B[e],g,:]
segment-MEAN over destination A -> softmax over g -> att @ vh -> Wc projection.

Two algebraic simplifications:
1) within a segment (fixed destination node n=A[e]) qh[n] is constant, so
       sums[n,h,g] = qh[n,h,:] . S[n,g,:],
       S[n,g,:]    = sum_{e:A[e]=n} kh[B[e],g,:]
   -- the [E,H,H] per-edge score tensor is never materialized.
2) the k-projection commutes with the segment sum:
       S = (sum_e one_hot(A)^T k[B[e]]) @ Wk^T + cnt * bk
   -- so the device only ever sees RAW k rows for its edge slice (the
   sharding step hands each core its pre-sliced per-edge k features), and
   projects AFTER aggregation: same FLOPs as projecting k once.

Sharding: nodes (rows) are sharded across the 8 cores; edges are bucketed by
the core that owns their destination node A[e] and sorted by A (host-side =
the sharding step), so no inter-core reduction is needed and no gather DMAs
are needed on device -- the per-edge k rows stream in dense order.

Per core, per 128-node block b:
  - stream K_edges tiles [128 edges, 256] (bf16)
  - one-hot(A_local) matmuls on TensorE accumulate U = O^T K_edges in PSUM
  - project: S = U @ Wk^T + cnt x bk   (PE, after a PE transpose of U)
  - DVE/ACT: score mul+reduce, exp(scale=1/cnt) softmax, V-phase
  - PE transpose + Wc matmul + bias, DMA out
"""

import numpy as np
import ml_dtypes

import concourse.bass as bass
import concourse.mybir as mybir
import concourse.tile as tile
from concourse.bass_utils import run_bass_kernel_spmd
from concourse.masks import make_identity

# ---------------------------------------------------------------- constants
NCORES = 8
N_NODES = 50000
EMB = 256
H = 8
D = 32
P = 128

NPC = N_NODES // NCORES          # 6250 nodes per core
NB = (NPC + P - 1) // P          # 49 blocks of 128 nodes per core
NPC_PAD = NB * P                 # 6272

FP = mybir.dt.float32
BF = mybir.dt.bfloat16


# ------------------------------------------------------- sync-wait splitting
# The staged walrus accepts only ONE sync-wait command per instruction.
# Tile attaches several waits to some instructions.  Post-pass: hoist all but
# one wait of each over-limit instruction onto same-engine Drain carriers
# placed immediately before it (engine streams execute in block order, so
# "all waits hold before the instruction runs" is preserved).
_WS_COUNTER = [0]


def _split_sync_waits(nc, maxw=1):
    for f in nc.m.functions:
        for blk in f.blocks:
            insts = blk.instructions
            out = []
            changed = False
            for ins in insts:
                si = ins.sync_info
                if si is not None and len(si.on_wait) > maxw:
                    waits = list(si.on_wait)
                    k = len(waits) - maxw
                    for i in range(0, k, maxw):
                        _WS_COUNTER[0] += 1
                        d = mybir.InstDrain(
                            name=f"I-wsplit-{_WS_COUNTER[0]}", ins=[], outs=[]
                        )
                        d.engine = ins.engine
                        d.sync_info = mybir.SyncInfo(
                            on_wait=waits[i : i + maxw], on_update=[]
                        )
                        out.append(d)
                    si.on_wait = waits[k:]
                    changed = True
                out.append(ins)
            if changed:
                blk.instructions = out


# ------------------------------------------------------------- device kernel
def build_nc(tiles_per_block, bf16=True, split_waits=True):
    """Build the SPMD Bass module. tiles_per_block[b] = edge tiles in block b
    (identical across cores; per-core edge data is padded to this)."""
    ET = int(sum(tiles_per_block))
    TMAX = int(max(tiles_per_block))
    KD = BF if bf16 else FP          # edge-feature / one-hot / Wk dtype
    CD = BF if bf16 else FP          # phase-C mul operand dtype

    nc = bass.Bass("TRN2", target_bir_lowering=False, debug=False,
                   num_devices=NCORES)

    # per-core inputs
    qT = nc.dram_tensor("qT", [EMB, NPC_PAD], FP, kind="ExternalInput")
    vT = nc.dram_tensor("vT", [EMB, NPC_PAD], FP, kind="ExternalInput")
    ke_d = nc.dram_tensor("ke", [ET * P, EMB], KD, kind="ExternalInput")
    WqT = nc.dram_tensor("WqT", [EMB, EMB], FP, kind="ExternalInput")
    WkT = nc.dram_tensor("WkT", [EMB, EMB], KD, kind="ExternalInput")
    WvT = nc.dram_tensor("WvT", [EMB, EMB], FP, kind="ExternalInput")
    WcT = nc.dram_tensor("WcT", [EMB, EMB], FP, kind="ExternalInput")
    bq = nc.dram_tensor("bq", [1, EMB], FP, kind="ExternalInput")
    bk = nc.dram_tensor("bk", [1, EMB], KD, kind="ExternalInput")
    bv = nc.dram_tensor("bv", [1, EMB], FP, kind="ExternalInput")
    bc = nc.dram_tensor("bc", [1, EMB], FP, kind="ExternalInput")
    aloc_d = nc.dram_tensor("aloc", [P, ET], FP, kind="ExternalInput")
    cnt_d = nc.dram_tensor("cnt", [1, NPC_PAD], KD, kind="ExternalInput")
    invc_d = nc.dram_tensor("invc", [P, NB], FP, kind="ExternalInput")

    out_d = nc.dram_tensor("out_shard", [NPC_PAD, EMB], FP, kind="ExternalOutput")

    with tile.TileContext(nc) as tc:
        with (
            tc.tile_pool(name="const", bufs=1) as cp,
            tc.tile_pool(name="work", bufs=3) as wp,
            tc.tile_pool(name="kep", bufs=2) as kp,
            tc.tile_pool(name="psum", bufs=2, space="PSUM") as pp,
            tc.tile_pool(name="psum1", bufs=1, space="PSUM") as pp1,
        ):
            # ---------------- constants
            iota_i = cp.tile([P, P], mybir.dt.int32)
            nc.gpsimd.iota(iota_i[:], pattern=[[1, P]], base=0, channel_multiplier=0)
            iota_c = cp.tile([P, P], KD)
            nc.vector.tensor_copy(iota_c[:], iota_i[:])
            ident = cp.tile([P, P], FP)
            make_identity(nc, ident[:])
            ident_k = cp.tile([P, P], KD)
            nc.vector.tensor_copy(ident_k[:], ident[:])
            ones1 = cp.tile([1, P], FP)
            nc.vector.memset(ones1[:], 1.0)

            wtiles = {}
            for nm, t, dt_ in (("Wq", WqT, FP), ("Wk", WkT, KD),
                               ("Wv", WvT, FP), ("Wc", WcT, FP)):
                a = cp.tile([P, EMB], dt_, tag=f"{nm}a")
                b = cp.tile([P, EMB], dt_, tag=f"{nm}b")
                nc.sync.dma_start(a[:], t[0:P, :])
                nc.sync.dma_start(b[:], t[P:EMB, :])
                wtiles[nm] = (a, b)
            btiles = {}
            for nm, t, dt_ in (("bq", bq, FP), ("bk", bk, KD),
                               ("bv", bv, FP), ("bc", bc, FP)):
                s = cp.tile([1, EMB], dt_, tag=nm)
                nc.sync.dma_start(s[:], t[:])
                btiles[nm] = s

            aloc_sb = cp.tile([P, ET], FP)
            nc.sync.dma_start(aloc_sb[:], aloc_d[:])
            cnt_sb = cp.tile([1, NPC_PAD], KD)
            nc.sync.dma_start(cnt_sb[:], cnt_d[:])
            invc_sb = cp.tile([P, NB], FP)
            nc.sync.dma_start(invc_sb[:], invc_d[:])

            wka, wkb = wtiles["Wk"]
            wqa, wqb = wtiles["Wq"]
            wva, wvb = wtiles["Wv"]
            wca, wcb = wtiles["Wc"]

            # ---------------- main loop over node blocks
            t0 = 0
            for b in range(NB):
                T = int(tiles_per_block[b])

                # projections qh, vh for this block -> one PSUM bank [P, 512]
                qta = wp.tile([P, P], FP, tag="qta")
                qtb = wp.tile([P, P], FP, tag="qtb")
                vta = wp.tile([P, P], FP, tag="vta")
                vtb = wp.tile([P, P], FP, tag="vtb")
                nc.sync.dma_start(qta[:], qT[0:P, b * P:(b + 1) * P])
                nc.sync.dma_start(qtb[:], qT[P:EMB, b * P:(b + 1) * P])
                nc.sync.dma_start(vta[:], vT[0:P, b * P:(b + 1) * P])
                nc.sync.dma_start(vtb[:], vT[P:EMB, b * P:(b + 1) * P])
                ps_qv = pp.tile([P, 2 * EMB], FP, space="PSUM", tag="qv")
                nc.tensor.matmul(out=ps_qv[:, 0:EMB], lhsT=qta[:], rhs=wqa[:], start=True, stop=False)
                nc.tensor.matmul(out=ps_qv[:, 0:EMB], lhsT=qtb[:], rhs=wqb[:], start=False, stop=False)
                nc.tensor.matmul(out=ps_qv[:, 0:EMB], lhsT=ones1[:], rhs=btiles["bq"][:], start=False, stop=True)
                nc.tensor.matmul(out=ps_qv[:, EMB:2 * EMB], lhsT=vta[:], rhs=wva[:], start=True, stop=False)
                nc.tensor.matmul(out=ps_qv[:, EMB:2 * EMB], lhsT=vtb[:], rhs=wvb[:], start=False, stop=False)
                nc.tensor.matmul(out=ps_qv[:, EMB:2 * EMB], lhsT=ones1[:], rhs=btiles["bv"][:], start=False, stop=True)
                qv_sb = wp.tile([P, 2 * EMB], CD, tag="qv_sb")
                nc.scalar.copy(qv_sb[:], ps_qv[:])
                qh_sb = qv_sb[:, 0:EMB]
                vh_sb = qv_sb[:, EMB:2 * EMB]

                # stream this block's edge k-rows: [T*128, 256] -> [128, T, 256]
                ke = kp.tile([P, TMAX, EMB], KD, tag="ke")
                nc.sync.dma_start(
                    ke[:, 0:T, :],
                    ke_d[t0 * P:(t0 + T) * P, :].rearrange("(t p) c -> p t c", p=P),
                )

                # U = sum_t one_hot(A_local)^T @ K_tile   (PSUM, fp32)
                ps_u = pp.tile([P, EMB], FP, space="PSUM", tag="U")
                for t in range(T):
                    oh = wp.tile([P, P], KD, tag="oh")
                    nc.vector.tensor_scalar(
                        out=oh[:], in0=iota_c[:], scalar1=aloc_sb[:, t0 + t:t0 + t + 1],
                        scalar2=None, op0=mybir.AluOpType.is_equal,
                    )
                    nc.tensor.matmul(out=ps_u[:], lhsT=oh[:], rhs=ke[:, t, :],
                                     start=(t == 0), stop=(t == T - 1))
                t0 += T

                u_sb = wp.tile([P, EMB], KD, tag="u_sb")
                nc.scalar.copy(u_sb[:], ps_u[:])

                # transpose U and project: S = U @ Wk^T + cnt x bk
                uT_a = wp.tile([P, P], KD, tag="uTa")
                uT_b = wp.tile([P, P], KD, tag="uTb")
                for i, dst in enumerate((uT_a, uT_b)):
                    tpk = pp1.tile([P, P], KD, space="PSUM", tag="tp")
                    nc.tensor.transpose(tpk[:], u_sb[:, i * P:(i + 1) * P], ident_k[:])
                    nc.scalar.copy(dst[:], tpk[:])
                ps_s = pp.tile([P, EMB], FP, space="PSUM", tag="acc")
                nc.tensor.matmul(out=ps_s[:], lhsT=uT_a[:], rhs=wka[:], start=True, stop=False)
                nc.tensor.matmul(out=ps_s[:], lhsT=uT_b[:], rhs=wkb[:], start=False, stop=False)
                nc.tensor.matmul(out=ps_s[:], lhsT=cnt_sb[:, b * P:(b + 1) * P],
                                 rhs=btiles["bk"][:], start=False, stop=True)
                s_sb = wp.tile([P, EMB], CD, tag="s_sb")
                nc.scalar.copy(s_sb[:], ps_s[:])

                # score: sums[n,h,g] = sum_d qh[n,h,d] * S[n,g,d]
                prod = wp.tile([P, H, H, D], CD, tag="prod")
                nc.vector.tensor_tensor(
                    out=prod[:],
                    in0=qh_sb.rearrange("p (h d) -> p h d", h=H).unsqueeze(2).to_broadcast([P, H, H, D]),
                    in1=s_sb[:].rearrange("p (g d) -> p g d", g=H).unsqueeze(1).to_broadcast([P, H, H, D]),
                    op=mybir.AluOpType.mult,
                )
                sc = wp.tile([P, H * H], FP, tag="sc")
                nc.vector.tensor_reduce(out=sc[:], in_=prod[:],
                                        axis=mybir.AxisListType.X, op=mybir.AluOpType.add)
                # mean + exp (softmax numerator); scale = 1/max(cnt,1) per node
                ex = wp.tile([P, H * H], FP, tag="ex")
                nc.scalar.activation(out=ex[:], in_=sc[:],
                                     func=mybir.ActivationFunctionType.Exp,
                                     scale=invc_sb[:, b:b + 1])
                den = wp.tile([P, H], FP, tag="den")
                nc.vector.tensor_reduce(out=den[:],
                                        in_=ex[:].rearrange("p (h g) -> p h g", h=H),
                                        axis=mybir.AxisListType.X, op=mybir.AluOpType.add)
                rden = wp.tile([P, H], FP, tag="rden")
                nc.vector.reciprocal(rden[:], den[:])
                att = wp.tile([P, H * H], CD, tag="att")
                nc.vector.tensor_tensor(
                    out=att[:].rearrange("p (h g) -> p h g", h=H),
                    in0=ex[:].rearrange("p (h g) -> p h g", h=H),
                    in1=rden[:].unsqueeze(2).to_broadcast([P, H, H]),
                    op=mybir.AluOpType.mult,
                )

                # V phase: ov[n,(h,d)] = sum_g att[n,h,g] * vh[n,g,d]
                p2 = wp.tile([P, H, D, H], CD, tag="p2")
                nc.vector.tensor_tensor(
                    out=p2[:],
                    in0=att[:].rearrange("p (h g) -> p h g", h=H).unsqueeze(2).to_broadcast([P, H, D, H]),
                    in1=vh_sb.rearrange("p (g d) -> p d g", g=H).unsqueeze(1).to_broadcast([P, H, D, H]),
                    op=mybir.AluOpType.mult,
                )
                ov = wp.tile([P, EMB], FP, tag="ov")
                nc.vector.tensor_reduce(out=ov[:], in_=p2[:],
                                        axis=mybir.AxisListType.X, op=mybir.AluOpType.add)

                # transpose ov, project with Wc, add bias
                ovT_a = wp.tile([P, P], FP, tag="ovTa")
                ovT_b = wp.tile([P, P], FP, tag="ovTb")
                for i, dst in enumerate((ovT_a, ovT_b)):
                    tp = pp1.tile([P, P], FP, space="PSUM", tag="tp")
                    nc.tensor.transpose(tp[:], ov[:, i * P:(i + 1) * P], ident[:])
                    nc.scalar.copy(dst[:], tp[:])
                ps_f = pp.tile([P, EMB], FP, space="PSUM", tag="acc")
                nc.tensor.matmul(out=ps_f[:], lhsT=ovT_a[:], rhs=wca[:], start=True, stop=False)
                nc.tensor.matmul(out=ps_f[:], lhsT=ovT_b[:], rhs=wcb[:], start=False, stop=False)
                nc.tensor.matmul(out=ps_f[:], lhsT=ones1[:], rhs=btiles["bc"][:], start=False, stop=True)
                fin = wp.tile([P, EMB], FP, tag="fin_sb")
                nc.scalar.copy(fin[:], ps_f[:])
                nc.sync.dma_start(out_d[b * P:(b + 1) * P, :], fin[:])

    if split_waits:
        _split_sync_waits(nc)
    return nc


# --------------------------------------------------------------- host prep
def _prep(q, k, v, edge_index, Wq, bq, Wk, bk, Wv, bv, Wc, bc, bf16=True):
    A = np.asarray(edge_index[0], dtype=np.int64)
    B = np.asarray(edge_index[1], dtype=np.int64)
    order = np.argsort(A, kind="stable")
    A_s = A[order]
    B_s = B[order]

    core_lo = np.searchsorted(A_s, np.arange(NCORES) * NPC, side="left")
    core_hi = np.searchsorted(A_s, (np.arange(NCORES) + 1) * NPC, side="left")

    counts = np.zeros((NCORES, NB), dtype=np.int64)
    per_core = []
    for o in range(NCORES):
        a = A_s[core_lo[o]:core_hi[o]] - o * NPC
        bi = B_s[core_lo[o]:core_hi[o]]
        blk = a // P
        counts[o] = np.bincount(blk, minlength=NB)
        per_core.append((a, bi, np.searchsorted(blk, np.arange(NB + 1))))
    tiles_per_block = np.maximum(1, (counts.max(axis=0) + P - 1) // P).astype(int)
    ET = int(tiles_per_block.sum())

    k = np.asarray(k, dtype=np.float32)
    kdt = ml_dtypes.bfloat16 if bf16 else np.float32
    k_cast = k.astype(kdt)
    alocs, kes = [], []
    for o in range(NCORES):
        a, bvals, bounds = per_core[o]
        al = np.full((ET * P,), -1.0, dtype=np.float32)
        be = np.zeros((ET * P,), dtype=np.int64)
        mask = np.zeros((ET * P,), dtype=bool)
        pos = 0
        for blk in range(NB):
            lo, hi = bounds[blk], bounds[blk + 1]
            n = hi - lo
            al[pos:pos + n] = (a[lo:hi] - blk * P).astype(np.float32)
            be[pos:pos + n] = bvals[lo:hi]
            mask[pos:pos + n] = True
            pos += tiles_per_block[blk] * P
        ke = np.zeros((ET * P, EMB), dtype=kdt)
        ke[mask] = k_cast[be[mask]]
        kes.append(ke)
        alocs.append(np.ascontiguousarray(al.reshape(ET, P).T))

    cnt_nodes = np.bincount(A, minlength=N_NODES).astype(np.float32)
    invc_full = 1.0 / np.maximum(cnt_nodes, 1.0)
    invcs, cnts = [], []
    for o in range(NCORES):
        s = np.ones(NPC_PAD, dtype=np.float32)
        s[:NPC] = invc_full[o * NPC:(o + 1) * NPC]
        invcs.append(np.ascontiguousarray(s.reshape(NB, P).T))
        c = np.zeros(NPC_PAD, dtype=np.float32)
        c[:NPC] = cnt_nodes[o * NPC:(o + 1) * NPC]
        cnts.append(c.reshape(1, NPC_PAD).astype(kdt))

    q = np.asarray(q, dtype=np.float32)
    v = np.asarray(v, dtype=np.float32)
    qTs, vTs = [], []
    for o in range(NCORES):
        qs = np.zeros((EMB, NPC_PAD), dtype=np.float32)
        vs = np.zeros((EMB, NPC_PAD), dtype=np.float32)
        qs[:, :NPC] = q[o * NPC:(o + 1) * NPC].T
        vs[:, :NPC] = v[o * NPC:(o + 1) * NPC].T
        qTs.append(qs)
        vTs.append(vs)

    com = {
        "WqT": np.ascontiguousarray(np.asarray(Wq, np.float32).T),
        "WkT": np.ascontiguousarray(np.asarray(Wk, np.float32).T).astype(kdt),
        "WvT": np.ascontiguousarray(np.asarray(Wv, np.float32).T),
        "WcT": np.ascontiguousarray(np.asarray(Wc, np.float32).T),
        "bq": np.asarray(bq, np.float32).reshape(1, EMB),
        "bk": np.asarray(bk, np.float32).reshape(1, EMB).astype(kdt),
        "bv": np.asarray(bv, np.float32).reshape(1, EMB),
        "bc": np.asarray(bc, np.float32).reshape(1, EMB),
    }
    in_maps = []
    for o in range(NCORES):
        m = dict(com)
        m["qT"] = qTs[o]
        m["vT"] = vTs[o]
        m["ke"] = kes[o]
        m["aloc"] = alocs[o]
        m["cnt"] = cnts[o]
        m["invc"] = invcs[o]
        in_maps.append(m)
    return tiles_per_block.tolist(), in_maps


_LAST = {}


def kernel(q, k, v, edge_index, Wq, bq, Wk, bk, Wv, bv, Wc, bc, latent=None,
           _want_results=False, _trace=False, _bf16=True, _tmpdir=None):
    tiles_per_block, in_maps = _prep(q, k, v, edge_index,
                                     Wq, bq, Wk, bk, Wv, bv, Wc, bc, bf16=_bf16)
    key = (tuple(tiles_per_block), _bf16)
    if _LAST.get("key") != key:
        _LAST["nc"] = build_nc(tiles_per_block, bf16=_bf16)
        _LAST["key"] = key
    nc = _LAST["nc"]

    res = run_bass_kernel_spmd(nc, in_maps, core_ids=list(range(NCORES)),
                               trace=_trace, tmpdir=_tmpdir)
    out = np.empty((N_NODES, EMB), dtype=np.float32)
    for o in range(NCORES):
        out[o * NPC:(o + 1) * NPC] = res.results[o]["out_shard"][:NPC]
    if _want_results:
        return out, res
    return out

